# revision 36
# baseline (speedup 1.0000x reference)
"""Trainium2 Bass kernel for the nn_Coupling problem.

8-core data-parallel: 2 images per core, weights replicated. All matmuls in
float32r. The host folds weight-norm into the weights and pre-transposes them
into lhsT layouts; the device runs only the activation datapath.
"""
import math

import numpy as np

import concourse.bass as bass
import concourse.bacc as bacc
import concourse.bass_isa as bass_isa
import concourse.tile as tile
from concourse import mybir
from concourse.bass_utils import run_bass_kernel_spmd

F32 = mybir.dt.float32
F32R = mybir.dt.float32r
AF = mybir.ActivationFunctionType
ALU = mybir.AluOpType
AX = mybir.AxisListType

N_CORES = 8
B_PER = 2
ISCALE = 24 ** -0.5
TAPS = [(dy, dx) for dy in range(3) for dx in range(3)]


def _v(t, off, dims):
    """View on a tile: keep its partition dim, custom free dims, elem offset."""
    ap = t[:]
    return bass.AP(tensor=ap.tensor, offset=ap.offset + off, ap=[ap.ap[0]] + dims)


def _vn(t, nparts, off, dims):
    """Like _v but restrict partition count."""
    ap = t[:]
    return bass.AP(tensor=ap.tensor, offset=ap.offset + off,
                   ap=[[ap.ap[0][0], nparts]] + dims)


def _dram(handle, off, dims):
    return bass.AP(tensor=handle, offset=off, ap=dims)


def build_program():
    nc = bacc.Bacc(None, target_bir_lowering=False)

    d_xc = nc.dram_tensor("xc", [B_PER, 8192], F32, kind="ExternalInput")
    d_xi = nc.dram_tensor("xi", [B_PER, 8192], F32R, kind="ExternalInput")
    d_ldj = nc.dram_tensor("ldj_in", [1, B_PER], F32, kind="ExternalInput")
    d_ident = nc.dram_tensor("ident", [128, 128], F32, kind="ExternalInput")
    d_pe = nc.dram_tensor("pe_tm", [128, 768], F32, kind="ExternalInput")
    d_rwb = nc.dram_tensor("rwb", [128, 64], F32, kind="ExternalInput")
    d_vti = nc.dram_tensor("vt_init", [128, 1056], F32R, kind="ExternalInput")
    d_ones = nc.dram_tensor("ones_r", [1, 32], F32R, kind="ExternalInput")
    d_onesrow = nc.dram_tensor("ones_row", [1, 1156], F32R, kind="ExternalInput")
    d_zeros = nc.dram_tensor("zeros_r", [128, 1156], F32R, kind="ExternalInput")
    d_w_in = nc.dram_tensor("w_in", [8, 864], F32R, kind="ExternalInput")
    d_b_in = nc.dram_tensor("b_in", [96, 1], F32, kind="ExternalInput")
    d_w_out = nc.dram_tensor("w_out", [97, 7056], F32R, kind="ExternalInput")
    d_blk = []
    for bi in range(2):
        d_blk.append({
            "w_conv": nc.dram_tensor(f"w_conv{bi}", [96, 1728], F32R, kind="ExternalInput"),
            "b_conv": nc.dram_tensor(f"b_conv{bi}", [96, 1], F32, kind="ExternalInput"),
            "w_gate": nc.dram_tensor(f"w_gate{bi}", [96, 384], F32R, kind="ExternalInput"),
            "b_gate": nc.dram_tensor(f"b_gate{bi}", [96, 2], F32, kind="ExternalInput"),
            "w_qkv": nc.dram_tensor(f"w_qkv{bi}", [96, 352], F32R, kind="ExternalInput"),
            "w_ag": nc.dram_tensor(f"w_ag{bi}", [128, 192], F32R, kind="ExternalInput"),
            "b_ag": nc.dram_tensor(f"b_ag{bi}", [96, 2], F32, kind="ExternalInput"),
            "g_ln1": nc.dram_tensor(f"g_ln1_{bi}", [128, 96], F32, kind="ExternalInput"),
            "b_ln1": nc.dram_tensor(f"b_ln1_{bi}", [128, 96], F32, kind="ExternalInput"),
            "g_ln2": nc.dram_tensor(f"g_ln2_{bi}", [128, 96], F32, kind="ExternalInput"),
            "b_ln2": nc.dram_tensor(f"b_ln2_{bi}", [128, 96], F32, kind="ExternalInput"),
        })
    d_out = nc.dram_tensor("out", [B_PER, 8192], F32, kind="ExternalOutput")
    d_ldj_out = nc.dram_tensor("ldj_out", [1, B_PER], F32, kind="ExternalOutput")

    with tile.TileContext(nc) as tc:
        with tc.tile_pool(name="sing", bufs=1) as sing, \
             tc.tile_pool(name="pw", bufs=4) as pw, \
             tc.tile_pool(name="pr", bufs=3) as pr, \
             tc.tile_pool(name="px", bufs=2) as px, \
             tc.tile_pool(name="pk", bufs=2) as pk, \
             tc.tile_pool(name="pad", bufs=2) as padp, \
             tc.tile_pool(name="ptm", bufs=3) as ptm, \
             tc.tile_pool(name="pmx", bufs=5) as pmx, \
             tc.tile_pool(name="pth", bufs=2) as pth, \
             tc.tile_pool(name="prd", bufs=1) as prd, \
             tc.tile_pool(name="psm", bufs=2) as psm, \
             tc.tile_pool(name="psS", bufs=2, space="PSUM") as psS, \
             tc.tile_pool(name="psA", bufs=3, space="PSUM") as psA, \
             tc.tile_pool(name="psB", bufs=1, space="PSUM") as psB:

            def T4(shape=(128, 1024)):
                return pw.tile(list(shape), F32, tag="t4", name="t4")

            def R4(shape=(128, 1024)):
                return pr.tile(list(shape), F32R, tag="r4", name="r4")

            def SM(shape, tag):
                return psm.tile(list(shape), F32, tag=tag, name=tag)

            def load(pool, dram, shape, dtype, tag):
                t = pool.tile(shape, dtype, tag=tag, name=tag)
                nc.gpsimd.dma_start(t[:], dram[:])
                return t

            ident = load(sing, d_ident, [128, 128], F32, "ident")
            pe_tm = load(sing, d_pe, [128, 768], F32, "pe_tm")
            rwb = load(sing, d_rwb, [128, 64], F32, "rwb")
            vt = load(sing, d_vti, [128, 1056], F32R, "vt")
            ones_r = load(sing, d_ones, [1, 32], F32R, "ones_r")
            ones_row = load(sing, d_onesrow, [1, 1156], F32R, "ones_row")
            w_in = load(sing, d_w_in, [8, 864], F32R, "w_in")
            b_in = load(sing, d_b_in, [96, 1], F32, "b_in")
            w_out = load(sing, d_w_out, [97, 7056], F32R, "w_out")
            blk = []
            for bi in range(2):
                bw = {}
                for nm, shape, dt in (
                    ("w_conv", [96, 1728], F32R), ("b_conv", [96, 1], F32),
                    ("w_gate", [96, 384], F32R), ("b_gate", [96, 2], F32),
                    ("w_qkv", [96, 352], F32R), ("w_ag", [128, 192], F32R),
                    ("b_ag", [96, 2], F32), ("g_ln1", [128, 96], F32),
                    ("b_ln1", [128, 96], F32), ("g_ln2", [128, 96], F32),
                    ("b_ln2", [128, 96], F32),
                ):
                    bw[nm] = load(sing, d_blk[bi][nm], shape, dt, f"{nm}{bi}")
                blk.append(bw)
            eps_t = sing.tile([128, 1], F32, tag="epsv", name="epsv")
            nc.vector.memset(eps_t[:], 1e-5)
            ldj_in = load(sing, d_ldj, [1, B_PER], F32, "ldj_in")
            ldj_acc = sing.tile([1, B_PER], F32, tag="ldjacc", name="ldjacc")

            def zero_r(ap):
                # zero an f32r region via DMA from the host zeros constant
                # (Memset/MemsetZero don't support f32r through walrus)
                nc.sync.dma_start(
                    ap, bass.AP(tensor=d_zeros, offset=0,
                                ap=[[1156, ap.ap[0][1]]] + list(ap.ap[1:])))

            def conv_rhs(t, tap, n):
                dy, dx = tap
                return _v(t, dy * 34 + dx + n * 16 * 34, [[34, 16], [1, 32]])

            def border_zero(h):
                zero_r(h[:, 0:35])
                zero_r(h[:, 1121:1156])
                zero_r(_v(h, 33, [[34, 33], [1, 2]]))

            def concat_elu(x_ap, out_a, out_b):
                """out_a = elu(x), out_b = elu(-x); outputs may be strided APs.
                elu(x) = relu(x) + exp(min(x,0)) - 1; exps run in place."""
                mn = T4()
                nc.gpsimd.tensor_scalar_min(out=mn[:96, :], in0=x_ap, scalar1=0.0)
                nc.scalar.activation(out=mn[:96, :], in_=mn[:96, :], func=AF.Exp)
                mx = T4()
                nc.vector.tensor_scalar_max(out=mx[:96, :], in0=x_ap, scalar1=0.0)
                nc.vector.scalar_tensor_tensor(out=out_a, in0=mn[:96, :], scalar=-1.0,
                                               in1=mx[:96, :], op0=ALU.add, op1=ALU.add)
                r2 = T4()
                nc.gpsimd.tensor_sub(r2[:96, :], mx[:96, :], x_ap)
                nc.scalar.activation(out=mx[:96, :], in_=mx[:96, :], func=AF.Exp,
                                     scale=-1.0)
                nc.vector.scalar_tensor_tensor(out=out_b, in0=mx[:96, :], scalar=-1.0,
                                               in1=r2[:96, :], op0=ALU.add, op1=ALU.add)

            def layer_norm_chunk(src_ap, out_ap, gam, bet):
                """LN over free dim (96) of [128, 96] src; writes out_ap."""
                st = SM([128, 6], "bnst")
                nc.vector.bn_stats(out=st[:], in_=src_ap)
                mv = SM([128, 2], "bnmv")
                nc.vector.bn_aggr(out=mv[:], in_=st[:])
                sd = SM([128, 1], "bnsd")
                nc.scalar.activation(out=sd[:], in_=mv[:, 1:2], func=AF.Sqrt,
                                     bias=eps_t[:])
                rstd = SM([128, 1], "bnrs")
                nc.vector.reciprocal(rstd[:], sd[:])
                nc.vector.tensor_scalar(out=out_ap, in0=src_ap, scalar1=mv[:, 0:1],
                                        scalar2=rstd[:], op0=ALU.subtract, op1=ALU.mult)
                nc.vector.tensor_mul(out_ap, out_ap, gam[:])
                nc.vector.tensor_add(out_ap, out_ap, bet[:])

            for img in range(B_PER):
                # ---------------- conv_in ----------------
                xpad = padp.tile([8, 1156], F32R, tag="xp", name="xp")
                zero_r(xpad[:])
                nc.sync.dma_start(_v(xpad, 35, [[34, 32], [1, 32]]),
                                  _dram(d_xi, img * 8192, [[1024, 8], [1, 1024]]))
                x_fm = px.tile([96, 1024], F32, tag="xfm", name="xfm")
                for n in range(2):
                    ps = psA.tile([96, 512], F32, tag="acc", name="acc")
                    for t9, tap in enumerate(TAPS):
                        nc.tensor.matmul(ps[:], w_in[:, t9 * 96:(t9 + 1) * 96],
                                         conv_rhs(xpad, tap, n),
                                         start=(t9 == 0), stop=(t9 == 8))
                    nc.vector.tensor_scalar_add(out=x_fm[:, n * 512:(n + 1) * 512],
                                                in0=ps[:], scalar1=b_in[:])

                # ---------------- blocks ----------------
                x5_pad = None
                for bi in range(2):
                    bw = blk[bi]
                    # gated conv
                    h1a = padp.tile([96, 1156], F32R, tag="pad", name="pad")
                    h1b = padp.tile([96, 1156], F32R, tag="pad", name="pad")
                    border_zero(h1a)
                    border_zero(h1b)
                    concat_elu(x_fm[:], _v(h1a, 35, [[34, 32], [1, 32]]),
                               _v(h1b, 35, [[34, 32], [1, 32]]))
                    u = T4()
                    for n in range(2):
                        ps = psA.tile([96, 512], F32, tag="acc", name="acc")
                        for j in range(18):
                            tap, half = j // 2, j % 2
                            nc.tensor.matmul(
                                ps[:], bw["w_conv"][:, j * 96:(j + 1) * 96],
                                conv_rhs(h1a if half == 0 else h1b, TAPS[tap], n),
                                start=(j == 0), stop=(j == 17))
                        nc.vector.tensor_scalar_add(
                            out=u[:96, n * 512:(n + 1) * 512], in0=ps[:],
                            scalar1=bw["b_conv"][:])
                    h2a = R4()
                    h2b = R4()
                    concat_elu(u[:96, :], h2a[:96, :], h2b[:96, :])
                    psm_g = []
                    for m in range(2):
                        ps = psS.tile([96, 1024], F32, tag="sp", name="sp")
                        for n in range(2):
                            for half in range(2):
                                nc.tensor.matmul(
                                    ps[:, n * 512:(n + 1) * 512],
                                    bw["w_gate"][:, (m * 2 + half) * 96:(m * 2 + half + 1) * 96],
                                    (h2a if half == 0 else h2b)[:96, n * 512:(n + 1) * 512],
                                    start=(half == 0), stop=(half == 1))
                        psm_g.append(ps)
                    sg = T4()
                    nc.scalar.activation(out=sg[:96, :], in_=psm_g[1][:], func=AF.Sigmoid,
                                         bias=bw["b_gate"][:, 1:2])
                    ag = T4()
                    nc.vector.tensor_scalar_add(out=ag[:96, :], in0=psm_g[0][:],
                                                scalar1=bw["b_gate"][:, 0:1])
                    x2 = T4()
                    nc.vector.tensor_mul(x2[:96, :], ag[:96, :], sg[:96, :])
                    nc.vector.tensor_add(x2[:96, :], x2[:96, :], x_fm[:])

                    # LN1 token-major
                    xln_tm = ptm.tile([128, 768], F32, tag="tm", name="tm")
                    for c in range(8):
                        pt = psA.tile([128, 96], F32, tag="acc", name="acc")
                        nc.tensor.transpose(pt[:], x2[:96, c * 128:(c + 1) * 128],
                                            ident[:96, :96])
                        layer_norm_chunk(pt[:], xln_tm[:, c * 96:(c + 1) * 96],
                                         bw["g_ln1"], bw["b_ln1"])
                    xpe_tm = ptm.tile([128, 768], F32, tag="tm", name="tm")
                    nc.vector.tensor_add(xpe_tm[:], xln_tm[:], pe_tm[:])
                    xpe_fm = R4((96, 1024))
                    for c in range(8):
                        pt = psA.tile([96, 128], F32, tag="acc", name="acc")
                        nc.tensor.transpose(pt[:], xpe_tm[:, c * 96:(c + 1) * 96],
                                            ident[:128, :128])
                        nc.vector.tensor_copy(xpe_fm[:96, c * 128:(c + 1) * 128], pt[:])

                    # attention in-proj. PE operands need 32-aligned base
                    # partitions (0/32/64 only), so the host zero-pads the K/Q
                    # projection columns to put head h at out-row 32h; psum then
                    # already has the gapped layout. Head 3 (base 96 illegal for
                    # PE) additionally gets a base-0 copy in tile b.
                    kq = {}
                    for m, nm in ((0, "k"), (1, "q"), (2, "v")):
                        if nm == "v":
                            ps = psS.tile([96, 1024], F32, tag="sp", name="sp")
                            for n in range(2):
                                nc.tensor.matmul(
                                    ps[:, n * 512:(n + 1) * 512],
                                    bw["w_qkv"][:, 256:352],
                                    xpe_fm[:96, n * 512:(n + 1) * 512],
                                    start=True, stop=True)
                            vfull = T4()
                            nc.vector.tensor_copy(vfull[:96, :], ps[:])
                            kq[nm] = vfull
                            continue
                        ps = psS.tile([128, 1024], F32, tag="sp", name="sp")
                        for n in range(2):
                            nc.tensor.matmul(ps[:, n * 512:(n + 1) * 512],
                                             bw["w_qkv"][:, m * 128:(m + 1) * 128],
                                             xpe_fm[:96, n * 512:(n + 1) * 512],
                                             start=True, stop=True)
                        ta = pk.tile([128, 1024], F32R, tag="kq", name="kq")
                        nc.vector.tensor_copy(ta[:], ps[:])
                        tb = pk.tile([24, 1024], F32R, tag="kq3", name="kq3")
                        nc.vector.tensor_copy(tb[:], ps[96:120, :])
                        kq[nm] = (ta, tb)

                    def hsl(pair, h, c0, c1):
                        ta, tb = pair
                        if h < 3:
                            return ta[h * 32:h * 32 + 24, c0:c1]
                        return tb[0:24, c0:c1]
                    # lhsT per (kc, h) is [128, 33]: V^T in cols 0-23, zeros in
                    # 24-31, ones in col 32 -> softmax denominator lands on psum
                    # partition 32 (a legal DVE base partition). The static
                    # zeros/ones pattern is host-loaded once (vt_init); only the
                    # V^T columns are rewritten per block.
                    for kc in range(8):
                        pt = psA.tile([128, 96], F32, tag="acc", name="acc")
                        nc.tensor.transpose(pt[:], kq["v"][:96, kc * 128:(kc + 1) * 128],
                                            ident[:96, :96])
                        nc.vector.tensor_copy(
                            _v(vt, kc * 132, [[33, 4], [1, 24]]),
                            pt[:].rearrange("p (h d) -> p h d", h=4))
                    att32 = pr.tile([128, 1024], F32R, tag="r4", name="r4")
                    for h in range(4):
                        av0 = psA.tile([33, 512], F32, tag="acc", name="acc")
                        av1 = psA.tile([33, 512], F32, tag="acc", name="acc")
                        for kc in range(8):
                            sp = psS.tile([128, 1024], F32, tag="sp", name="sp")
                            for n in range(2):
                                nc.tensor.matmul(
                                    sp[:, n * 512:(n + 1) * 512],
                                    hsl(kq["k"], h, kc * 128, (kc + 1) * 128),
                                    hsl(kq["q"], h, n * 512, (n + 1) * 512),
                                    start=True, stop=True)
                            ptile = R4()
                            nc.scalar.activation(out=ptile[:], in_=sp[:], func=AF.Exp,
                                                 scale=ISCALE)
                            vts = vt[:, kc * 132 + h * 33:kc * 132 + (h + 1) * 33]
                            nc.tensor.matmul(av0[:], vts, ptile[:, 0:512],
                                             start=(kc == 0), stop=(kc == 7))
                            nc.tensor.matmul(av1[:], vts, ptile[:, 512:1024],
                                             start=(kc == 0), stop=(kc == 7))
                        hs = h * 32
                        for n, av in ((0, av0), (1, av1)):
                            rden = prd.tile([1, 512], F32R, tag="rden", name="rden",
                                            bufs=1)
                            with nc.allow_low_precision(reason="f32r 1/den"):
                                nc.vector.reciprocal(rden[:], av[32:33, :])
                            bc = psB.tile([32, 512], F32, tag="bc", name="bc")
                            nc.tensor.matmul(bc[:], ones_r[:], rden[:],
                                             start=True, stop=True)
                            bcs = prd.tile([32, 512], F32, tag="bcs", name="bcs", bufs=1)
                            nc.vector.tensor_copy(bcs[:], bc[:])
                            # rows 24-31 of av are zeros (zero lhsT cols), so
                            # this also clears the gap rows of att32.
                            nc.vector.tensor_mul(
                                att32[hs:hs + 32, n * 512:(n + 1) * 512],
                                av[0:32, :], bcs[:])
                    # gate + glu
                    psg = []
                    for m in range(2):
                        ps = psS.tile([96, 1024], F32, tag="sp", name="sp")
                        for n in range(2):
                            nc.tensor.matmul(ps[:, n * 512:(n + 1) * 512],
                                             bw["w_ag"][:, m * 96:(m + 1) * 96],
                                             att32[:, n * 512:(n + 1) * 512],
                                             start=True, stop=True)
                        psg.append(ps)
                    sga = T4()
                    nc.scalar.activation(out=sga[:96, :], in_=psg[1][:], func=AF.Sigmoid,
                                         bias=bw["b_ag"][:, 1:2])
                    aga = T4()
                    nc.vector.tensor_scalar_add(out=aga[:96, :], in0=psg[0][:],
                                                scalar1=bw["b_ag"][:, 0:1])
                    glu = T4()
                    nc.vector.tensor_mul(glu[:96, :], aga[:96, :], sga[:96, :])

                    # residual (tm) + LN2
                    x5_tm = ptm.tile([128, 768], F32, tag="tm", name="tm")
                    for c in range(8):
                        pt = psA.tile([128, 96], F32, tag="acc", name="acc")
                        nc.tensor.transpose(pt[:], glu[:96, c * 128:(c + 1) * 128],
                                            ident[:96, :96])
                        x4c = SM([128, 96], "x4c")
                        nc.vector.tensor_add(x4c[:], pt[:],
                                             xln_tm[:, c * 96:(c + 1) * 96])
                        layer_norm_chunk(x4c[:], x5_tm[:, c * 96:(c + 1) * 96],
                                         bw["g_ln2"], bw["b_ln2"])
                    if bi == 0:
                        nxt = px.tile([96, 1024], F32, tag="xfm", name="xfm")
                    else:
                        x5_pad = padp.tile([97, 1156], F32R, tag="pad", name="pad")
                        border_zero(x5_pad)
                        nc.sync.dma_start(x5_pad[96:97, :], ones_row[:])
                    for c in range(8):
                        pt = psA.tile([96, 128], F32, tag="acc", name="acc")
                        nc.tensor.transpose(pt[:], x5_tm[:, c * 96:(c + 1) * 96],
                                            ident[:128, :128])
                        if bi == 0:
                            nc.vector.tensor_copy(nxt[:, c * 128:(c + 1) * 128], pt[:])
                        else:
                            nc.vector.tensor_copy(
                                _vn(x5_pad, 96, 35 + 4 * c * 34, [[34, 4], [1, 32]]),
                                pt[:])
                    if bi == 0:
                        x_fm = nxt

                # ---------------- conv_out + mixture ----------------
                # conv_out computed transposed: psum [128 pix, 784 params] per
                # spatial chunk, with x5_pad (plus a ones row carrying the
                # bias) as the stationary operand and w_out moving.
                xc_fm = T4((8, 1024))
                nc.sync.dma_start(xc_fm[:8, :],
                                  _dram(d_xc, img * 8192, [[1024, 8], [1, 1024]]))
                xct = SM([128, 64], "xct")
                ths = [pth.tile([128, 3136], F32, tag="th", name="th")
                       for _ in range(2)]
                for c in range(8):
                    pt = psA.tile([128, 8], F32, tag="acc", name="acc")
                    nc.tensor.transpose(pt[:], xc_fm[:8, c * 128:(c + 1) * 128],
                                        ident[:8, :8])
                    nc.vector.tensor_copy(xct[:, c * 8:c * 8 + 8], pt[:])
                    lhsTs = []
                    for t9, (dy, dx) in enumerate(TAPS):
                        xw = pr.tile([97, 128], F32R, tag="xw", name="xw", bufs=9)
                        nc.gpsimd.tensor_copy(
                            xw[:], _vn(x5_pad, 97,
                                       4 * c * 34 + dy * 34 + dx,
                                       [[34, 4], [1, 32]]))
                        lhsTs.append(xw)
                    for nh in range(2):
                        psc = psA.tile([128, 392], F32, tag="acc", name="acc")
                        for t9 in range(9):
                            nc.tensor.matmul(
                                psc[:], lhsTs[t9][:],
                                w_out[:, t9 * 784 + nh * 392:t9 * 784 + (nh + 1) * 392],
                                start=(t9 == 0), stop=(t9 == 8))
                        nc.vector.tensor_copy(
                            ths[nh][:, c * 392:(c + 1) * 392], psc[:])

                outc_all = SM([128, 64], "outc")
                ldrow = SM([128, 2], "ldrow")
                for half in range(2):
                    th = ths[half]

                    def GV(p0, w):
                        return _v(th, p0, [[392, 8], [98, 4], [1, w]])

                    def GV1(p0):
                        return _v(th, p0, [[392, 8], [98, 4]])

                    def HV(t):
                        return _v(t, 4 * half, [[8, 8], [1, 4]])

                    def G3(t):
                        return t[:].rearrange("p (g w) -> p c i w", c=8, i=4, w=32) \
                            if False else t[:].rearrange("p (c i w) -> p c i w", c=8, i=4)

                    def G2(t):
                        return t[:].rearrange("p (c i) -> p c i", c=8)

                    def MX():
                        return pmx.tile([128, 1024], F32, tag="mx", name="mx")

                    ls = MX()
                    nc.vector.tensor_scalar_max(out=G3(ls), in0=GV(66, 32), scalar1=-7.0)
                    einv = MX()
                    nc.scalar.activation(out=einv[:], in_=ls[:], func=AF.Exp, scale=-1.0)
                    xcb = MX()
                    nc.gpsimd.tensor_copy(G3(xcb),
                                          _v(xct, 4 * half, [[8, 8], [1, 4], [0, 32]]))
                    negz = MX()
                    nc.gpsimd.tensor_sub(G3(negz), GV(34, 32), G3(xcb))
                    nc.vector.tensor_mul(negz[:], negz[:], einv[:])
                    # softplus(negz)
                    spm = MX()
                    nc.vector.tensor_scalar_min(out=spm[:], in0=negz[:], scalar1=30.0)
                    spe = MX()
                    nc.scalar.activation(out=spe[:], in_=spm[:], func=AF.Exp)
                    spl = MX()
                    nc.scalar.activation(out=spl[:], in_=spe[:], func=AF.Ln, bias=1.0)
                    spr = MX()
                    nc.gpsimd.tensor_scalar(out=spr[:], in0=negz[:], scalar1=-30.0,
                                            scalar2=0.0, op0=ALU.add, op1=ALU.max)
                    sp_t = MX()
                    nc.vector.tensor_add(sp_t[:], spl[:], spr[:])
                    carg = MX()
                    nc.vector.tensor_sub(G3(carg), GV(2, 32), G3(sp_t))
                    parg = MX()
                    nc.vector.scalar_tensor_tensor(out=parg[:], in0=sp_t[:], scalar=-2.0,
                                                   in1=negz[:], op0=ALU.mult, op1=ALU.add)
                    nc.gpsimd.tensor_sub(parg[:], parg[:], ls[:])
                    nc.gpsimd.tensor_add(G3(parg), G3(parg), GV(2, 32))
                    lse = {}
                    for nm, src in (("c", carg), ("p", parg), ("pi", None)):
                        if src is None:
                            e = MX()
                            nc.scalar.activation(out=G3(e), in_=GV(2, 32), func=AF.Exp)
                        else:
                            e = src
                            nc.scalar.activation(out=e[:], in_=src[:], func=AF.Exp)
                        s_ = SM([128, 32], f"lse_{nm}")
                        nc.vector.tensor_reduce(out=G2(s_), in_=G3(e), axis=AX.X,
                                                op=ALU.add)
                        ln_ = SM([128, 32], f"lsel_{nm}")
                        nc.scalar.activation(out=ln_[:], in_=s_[:], func=AF.Ln)
                        lse[nm] = ln_
                    logcdf = SM([128, 32], "logcdf")
                    nc.vector.tensor_sub(logcdf[:], lse["c"][:], lse["pi"][:])
                    cdf = SM([128, 32], "cdf")
                    nc.scalar.activation(out=cdf[:], in_=logcdf[:], func=AF.Exp)
                    rec = SM([128, 32], "rec")
                    nc.vector.reciprocal(rec[:], cdf[:])
                    r1 = SM([128, 32], "r1")
                    nc.vector.tensor_scalar(out=r1[:], in0=rec[:], scalar1=-1.0,
                                            scalar2=1e-22, op0=ALU.add, op1=ALU.max)
                    lnr = SM([128, 32], "lnr")
                    nc.scalar.activation(out=lnr[:], in_=r1[:], func=AF.Ln)
                    omc = SM([128, 32], "omc")
                    nc.vector.tensor_scalar(out=omc[:], in0=cdf[:], scalar1=-1.0,
                                            scalar2=1.0, op0=ALU.mult, op1=ALU.add)
                    nc.vector.tensor_scalar_max(out=omc[:], in0=omc[:], scalar1=1e-22)
                    lnomc = SM([128, 32], "lnomc")
                    nc.scalar.activation(out=lnomc[:], in_=omc[:], func=AF.Ln)
                    tha = SM([128, 32], "tha")
                    nc.scalar.activation(out=G2(tha), in_=GV1(0), func=AF.Tanh)
                    a_t = SM([128, 32], "a_t")
                    nc.vector.tensor_mul(G2(a_t), G2(tha), HV(rwb))
                    ea = SM([128, 32], "ea")
                    nc.scalar.activation(out=ea[:], in_=a_t[:], func=AF.Exp)
                    zt = SM([128, 32], "zt")
                    nc.vector.tensor_sub(G2(zt), GV1(1), G2(lnr))
                    nc.vector.tensor_mul(HV(outc_all), G2(zt), G2(ea))
                    ldc = SM([128, 32], "ldc")
                    nc.vector.tensor_sub(ldc[:], lse["p"][:], lse["c"][:])
                    nc.gpsimd.tensor_sub(ldc[:], ldc[:], lnomc[:])
                    nc.gpsimd.tensor_add(ldc[:], ldc[:], a_t[:])
                    hr = SM([128, 1], f"hr{half}")
                    nc.vector.tensor_reduce(out=hr[:], in_=ldc[:], axis=AX.X, op=ALU.add)
                    nc.vector.tensor_copy(ldrow[:, half:half + 1], hr[:])

                ldsum = SM([128, 1], "ldsum")
                nc.vector.tensor_add(ldsum[:], ldrow[:, 0:1], ldrow[:, 1:2])
                ldar = SM([128, 1], "ldar")
                nc.gpsimd.partition_all_reduce(ldar[:], ldsum[:], channels=128,
                                               reduce_op=bass_isa.ReduceOp.add)
                nc.vector.tensor_copy(ldj_acc[:, img:img + 1], ldar[0:1, :])

                outfm = T4((8, 1024))
                for c in range(8):
                    pt = psA.tile([8, 128], F32, tag="acc", name="acc")
                    nc.tensor.transpose(pt[:], outc_all[:, c * 8:c * 8 + 8],
                                        ident[:128, :128])
                    nc.vector.tensor_copy(outfm[:8, c * 128:(c + 1) * 128], pt[:])
                nc.sync.dma_start(_dram(d_out, img * 8192, [[1024, 8], [1, 1024]]),
                                  outfm[:8, :])

            ldj_fin = sing.tile([1, B_PER], F32, tag="ldjfin", name="ldjfin")
            nc.vector.tensor_add(ldj_fin[:], ldj_acc[:], ldj_in[:])
            nc.sync.dma_start(d_ldj_out[:], ldj_fin[:])

    nc.finalize()
    return nc


# ======================= host side =======================

def _wn_rows(v, g):
    v = np.asarray(v, np.float64)
    g = np.asarray(g, np.float64)
    n = np.sqrt((v.reshape(v.shape[0], -1) ** 2).sum(1))
    shape = (v.shape[0],) + (1,) * (v.ndim - 1)
    return (g.reshape(shape) * v / n.reshape(shape)).astype(np.float32)


def _pos_enc_tm():
    position = np.arange(1024, dtype=np.float32)
    inv = np.exp(np.arange(48, dtype=np.float32) * (-math.log(10000.0) / 47.0))
    st = position[:, None] * inv[None, :]
    pe = np.concatenate([np.sin(st), np.cos(st)], axis=1).astype(np.float32)
    return pe.reshape(8, 128, 96).transpose(1, 0, 2).reshape(128, 768).copy()


def _prep_weights(params):
    w = {}
    wn = _wn_rows(params["in_v"], params["in_g"])  # [96,8,3,3]
    # w_in9[i, 96*tap + o] = wn[o, i, tap]
    w["w_in"] = wn.transpose(1, 2, 3, 0).reshape(8, 864).copy()
    w["b_in"] = np.asarray(params["in_b"], np.float32).reshape(96, 1).copy()
    wn = _wn_rows(params["out_v"], params["out_g"])
    w_out = np.zeros((97, 7056), np.float32)
    w_out[:96] = (wn.reshape(98, 8, 96, 3, 3)
                  .transpose(2, 3, 4, 1, 0).reshape(96, 7056))
    # bias rides on the ones row of the stationary operand, tap 0 only
    w_out[96, 0:784] = np.asarray(params["out_b"], np.float32)
    w["w_out"] = w_out
    w["ones_row"] = np.ones((1, 1156), np.float32)
    for bi, bp in enumerate(params["blocks"]):
        cp, ap = bp["conv"], bp["attn"]
        wn = _wn_rows(cp["conv_v"], cp["conv_g"])
        w[f"w_conv{bi}"] = (wn.reshape(96, 2, 96, 3, 3)
                            .transpose(2, 3, 4, 1, 0).reshape(96, 1728).copy())
        w[f"b_conv{bi}"] = np.asarray(cp["conv_b"], np.float32).reshape(96, 1).copy()
        wn = _wn_rows(cp["gate_v"], cp["gate_g"]).reshape(192, 192)
        w[f"w_gate{bi}"] = (wn.reshape(2, 96, 2, 96)
                            .transpose(3, 0, 2, 1).reshape(96, 384).copy())
        w[f"b_gate{bi}"] = np.asarray(cp["gate_b"], np.float32).reshape(2, 96).T.copy()
        wqkv = _wn_rows(ap["in_v"], ap["in_g"]).T  # [96 in, 288 out] K|V|Q
        wq = np.zeros((96, 352), np.float32)
        for h in range(4):
            wq[:, h * 32:h * 32 + 24] = wqkv[:, h * 24:(h + 1) * 24]            # K
            wq[:, 128 + h * 32:128 + h * 32 + 24] = wqkv[:, 192 + h * 24:192 + (h + 1) * 24]  # Q
        wq[:, 256:352] = wqkv[:, 96:192]                                        # V
        w[f"w_qkv{bi}"] = wq
        wag = _wn_rows(ap["gate_v"], ap["gate_g"]).T  # [96, 192]
        wag128 = np.zeros((128, 192), np.float32)
        for h in range(4):
            wag128[h * 32:h * 32 + 24] = wag[h * 24:(h + 1) * 24]
        w[f"w_ag{bi}"] = wag128
        w[f"b_ag{bi}"] = np.asarray(ap["gate_b"], np.float32).reshape(2, 96).T.copy()
        w[f"g_ln1_{bi}"] = np.repeat(np.asarray(bp["ln1_g"], np.float32)[None], 128, 0)
        w[f"b_ln1_{bi}"] = np.repeat(np.asarray(bp["ln1_b"], np.float32)[None], 128, 0)
        w[f"g_ln2_{bi}"] = np.repeat(np.asarray(bp["ln2_g"], np.float32)[None], 128, 0)
        w[f"b_ln2_{bi}"] = np.repeat(np.asarray(bp["ln2_b"], np.float32)[None], 128, 0)
    rv = np.asarray(params["rescale_v"], np.float64).reshape(8)
    rg = np.asarray(params["rescale_g"], np.float64).reshape(8)
    rw = (rg * rv / np.abs(rv)).astype(np.float32)
    w["rwb"] = np.repeat(np.tile(rw, 8)[None, :], 128, 0).copy()
    w["zeros_r"] = np.zeros((128, 1156), np.float32)
    w["ones_r"] = np.ones((1, 32), np.float32)
    vti = np.zeros((128, 1056), np.float32)
    vti[:, 32::33] = 1.0
    w["vt_init"] = vti
    w["ident"] = np.eye(128, dtype=np.float32)
    w["pe_tm"] = _pos_enc_tm()
    return w


_PROG_CACHE = {}


def kernel(x, ldj, params):
    x = np.asarray(x, np.float32)
    ldj = np.asarray(ldj, np.float32)
    if "prog" not in _PROG_CACHE:
        _PROG_CACHE["prog"] = build_program()
    nc = _PROG_CACHE["prog"]

    w = _prep_weights(params)
    x_change, x_id = x[:, :8], x[:, 8:]
    in_maps = []
    for c in range(N_CORES):
        m = dict(w)
        sl = slice(c * B_PER, (c + 1) * B_PER)
        m["xc"] = x_change[sl].reshape(B_PER, 8192).copy()
        m["xi"] = x_id[sl].reshape(B_PER, 8192).copy()
        m["ldj_in"] = ldj[sl].reshape(1, B_PER).copy()
        in_maps.append(m)

    res = run_bass_kernel_spmd(nc, in_maps, core_ids=list(range(N_CORES)))

    out = np.empty((16, 16, 32, 32), np.float32)
    out[:, 8:] = x_id
    ldj_out = np.empty((16,), np.float32)
    for c in range(N_CORES):
        r = res.results[c]
        out[c * B_PER:(c + 1) * B_PER, :8] = r["out"].reshape(B_PER, 8, 32, 32)
        ldj_out[c * B_PER:(c + 1) * B_PER] = r["ldj_out"].reshape(B_PER)
    return out, ldj_out
